# revision 19
# baseline (speedup 1.0000x reference)
"""Trainium2 Bass kernel for masked attention scoring (sparse_attention).

Computes, per batch b:
    proj = y @ M^T                      # [B, D]
    eij  = tanh(einsum('bsd,bd->bs', x, proj))
    a    = exp(eij) * mask
    a    = a / (sum_s a + EPS)

Sharding: data-parallel over batch B=32 across 8 NeuronCores (4 batches
per core). M is replicated; all reductions stay local per shard.

Per-core device algorithm (memory-bound; floor = 19 MB HBM traffic at
~358 GB/s = ~53 us). Host marshalling ships x pre-transposed to
[B, D, S] f16 (plus y^T, M^T f16 as before), so the d-contraction runs
on the TensorEngine and the Vector/Scalar engines are nearly idle:
  - projT[e, b] is computed directly in transposed layout on PE:
    lhsT = M^T chunk [128d, 128e], rhs = y^T [128d, 4b], accumulated
    over the 8 d-chunks into one PSUM bank ([128, 8, 4] f32), then one
    f16 copy to SBUF. PE clock pre-warmed with dummy transposes.
  - main pass: stream xT in [128, 4, 2048] f16 tiles (2 MiB DMAs, 4 KiB
    per-partition lines). Per (batch, d-group, s-chunk): one matmul
    with the x tile as the stationary operand (lhsT [128d, 128s]) and
    rhs = projT[:, dc, b] [128, 1], accumulating eij[s, st] chains in
    PSUM over the 8 d-chunks. Two PSUM banks ping-pong across batches
    so the tanh extraction of batch b overlaps batch b+1's matmuls.
  - epilogue (s-on-partitions [128, BL, 16] layout): tanh fused into
    the PSUM->SBUF extraction on ScalarE, then exp, mask multiply,
    free-dim reduce, partition reduce + denominator broadcast via tiny
    TensorE matmuls, normalize, PE-transpose, one contiguous DMA out.
"""

import os
import sys

import numpy as np

for _p in ("/opt/trn_rl_repo",):
    if os.path.isdir(_p) and _p not in sys.path:
        sys.path.insert(0, _p)

B, S, D = 32, 2048, 1024
NCORES = 8
BL = B // NCORES        # batches per core
P = 128                 # SBUF partitions
J = S // P              # 16 s-chunks per batch
DC = D // P             # 8 d-chunks

_CACHE = {}


def _build():
    import concourse.bacc as bacc
    import concourse.bass as bass_mod
    import concourse.tile as tile
    from concourse import mybir
    from concourse.masks import make_identity
    from concourse.tile import add_dep_helper

    f32 = mybir.dt.float32
    f16 = mybir.dt.float16
    i32 = mybir.dt.int32

    nc = bacc.Bacc("TRN2", target_bir_lowering=False, debug=False,
                   num_devices=NCORES)

    x_ext = nc.dram_tensor("xT16", [BL, D, S], f16, kind="ExternalInput").ap()
    # y pre-swizzled on host to [p, dc, b] so the DMA is one 64 B line per
    # partition instead of 1024 8-byte descriptors
    y_ext = nc.dram_tensor("ysw16", [P, DC, BL], f16,
                           kind="ExternalInput").ap()
    mask_ext = nc.dram_tensor("mask", [BL, S], i32, kind="ExternalInput").ap()
    m_ext = nc.dram_tensor("MT16", [D, D], f16, kind="ExternalInput").ap()
    out_ext = nc.dram_tensor("out", [BL, S], f32, kind="ExternalOutput").ap()

    with tile.TileContext(nc) as tc:
        with (
            tc.tile_pool(name="consts", bufs=1) as consts,
            tc.tile_pool(name="psum_warm", bufs=1, space="PSUM") as psum_warm,
            tc.tile_pool(name="psum_pt", bufs=1, space="PSUM") as psum_pt,
            tc.tile_pool(name="psum_eij", bufs=2, space="PSUM") as psum_eij,
            tc.tile_pool(name="psum_small", bufs=1, space="PSUM") as psum_small_pool,
            tc.tile_pool(name="xpool", bufs=8) as xpool,
        ):
            identity16 = consts.tile([P, P], f16)
            make_identity(nc, identity16)
            identity32 = consts.tile([P, P], f32)
            make_identity(nc, identity32)
            ones_mat = consts.tile([P, P], f16)
            nc.vector.memset(ones_mat, 1.0)

            # ---- y ships pre-swizzled [p, dc, b] f16 from the host ----
            yT = consts.tile([P, DC, BL], f16)
            nc.gpsimd.dma_start(out=yT, in_=y_ext)

            # ---- M^T ships pre-transposed f16 from the host ----
            # mtsb[p_dd, dc, e] = M[e, dc*128+p_dd]; 4 chunks alternating
            # the two HWDGE rings so projT matmuls can chase the stream
            mtsb = consts.tile([P, DC, D], f16)
            m_src = m_ext.rearrange("(dc p) e -> p dc e", p=P)
            for mc in range(4):
                eng = nc.sync if mc % 2 == 0 else nc.scalar
                eng.dma_start(out=mtsb[:, 2 * mc:2 * mc + 2, :],
                              in_=m_src[:, 2 * mc:2 * mc + 2, :])

            # warm the PE clock (1.2 -> 2.4 GHz needs ~4us sustained)
            warm_ps = psum_warm.tile([P, P], f16, tag="warm", bufs=1)
            for _ in range(12):
                nc.tensor.transpose(warm_ps, identity16, identity16)

            # ---- upfront x DMA issue: all tiles in flight, both rings ----
            # tile sizes ramp up (fast pipeline start) and the last batch
            # uses 1 MiB tiles (fine-grained tail)
            # rings: "s"=sync HWDGE, "a"=ACT HWDGE. Batches 0-2 split
            # across both rings (parallel descriptor-gen, fast ramp); ALL
            # of batch 3 on sync, which drains first and then gets every
            # SDMA engine for the final tiles.
            GLISTS = [[1, 1, 2, 4], [4, 4], [4, 4], [4, 2, 1, 1]]
            RINGS = [["s", "a", "s", "a"], ["s", "a"], ["s", "a"],
                     ["s", "s", "s", "s"]]
            xtiles = []         # per batch: list of (g_size, dc0, tile)
            for b in range(BL):
                tiles_b = []
                dc0 = 0
                for gs, rg in zip(GLISTS[b], RINGS[b]):
                    xt = xpool.tile([P, gs, S], f16, tag=f"xt{gs}",
                                    bufs=sum(gl.count(gs) for gl in GLISTS),
                                    name=f"xt_{b}_{dc0}")
                    eng = nc.sync if rg == "s" else nc.scalar
                    eng.dma_start(
                        out=xt,
                        in_=x_ext[b, dc0 * P:(dc0 + gs) * P, :]
                        .rearrange("(g p) s -> p g s", p=P),
                    )
                    tiles_b.append((gs, dc0, xt))
                    dc0 += gs
                xtiles.append(tiles_b)

            # ---- projT[e, b] = sum_d M[e, d] y[b, d], e on partitions ----
            # lhsT = mtsb[:, dc, ec-slice] ([128 d, 128 e]), rhs = yT[:, dc, :]
            # dc-major so matmuls chase the 4 M chunks as they land
            pt_ps = psum_pt.tile([P, DC, BL], f32)
            proj_mms = []
            for dc in range(DC):
                for ec in range(DC):
                    proj_mms.append(nc.tensor.matmul(
                        pt_ps[:, ec, :],
                        lhsT=mtsb[:, dc, ec * P:(ec + 1) * P],
                        rhs=yT[:, dc, :],
                        start=(dc == 0 and ec == 0),
                        stop=(dc == DC - 1),
                    ))
            projT = consts.tile([P, DC, BL], f16)
            nc.vector.tensor_copy(projT, pt_ps)

            # ---- masks: one contiguous cast-DMA + PE transposes ----
            mk_nat = consts.tile([J, BL, P], f32)
            nc.gpsimd.dma_start(
                out=mk_nat,
                in_=mask_ext.rearrange("b (j p) -> j b p", p=P))
            mask_all = consts.tile([P, BL, J], f32)
            for b in range(BL):
                mk_ps = psum_small_pool.tile([P, J], f32, tag="small")
                mk_t = nc.tensor.transpose(mk_ps, mk_nat[:, b, :],
                                           identity32[:J, :J])
                add_dep_helper(mk_t.ins, proj_mms[-1].ins, sync=False,
                               reason="mask transposes after projT GEMM")
                nc.vector.tensor_copy(mask_all[:, b, :], mk_ps)

            # ---- main pass: eij[p, st] = x[b, st*128+p, :] . proj[b, :] ----
            # x tile is the stationary operand; rhs is projT column b.
            # Per-batch epilogue (incl. its own 8 KiB out-DMA) so only the
            # last batch's short chain trails the x stream.
            th = consts.tile([P, BL, J], f32)
            ex = consts.tile([P, BL, J], f32)
            au = consts.tile([P, BL, J], f32)
            an = consts.tile([P, BL, J], f32)
            cs = consts.tile([P, BL], f16)
            rec = consts.tile([P, BL], f32)
            prev_chain = None
            for b in range(BL):
                eij_ps = psum_eij.tile([P, J], f32, tag="eij")
                for gs, dc0, xt in xtiles[b]:
                    for g in range(gs):
                        dc = dc0 + g
                        for st in range(J):
                            # start=True clears has_written for the WHOLE
                            # bank, so only the first matmul per bank may
                            # set it; the other dc==0 writes then land on
                            # pending-zero bytes and overwrite correctly.
                            nc.tensor.matmul(
                                eij_ps[:, st:st + 1],
                                lhsT=xt[:, g, st * P:(st + 1) * P],
                                rhs=projT[:, dc, b:b + 1],
                                start=(dc == 0 and st == 0),
                                stop=(dc == DC - 1),
                            )
                # tanh fused into the PSUM evacuation; then exp + masked
                # row-sum per batch so only the global normalize trails the
                # stream. Explicit edges keep batch b's chain ahead of
                # batch b+1's in each engine's static order (ScalarE/
                # VectorE are strict FIFO - a misordered pair serializes
                # the whole pipeline behind the later batch's data).
                t_op = nc.scalar.activation(th[:, b, :], eij_ps,
                                            mybir.ActivationFunctionType.Tanh)
                e_op = nc.scalar.activation(ex[:, b, :], th[:, b, :],
                                            mybir.ActivationFunctionType.Exp)
                # au = ex * mask and cs = sum_j au in one DVE op
                s_op = nc.vector.scalar_tensor_tensor(
                    out=au[:, b, :],
                    in0=ex[:, b, :],
                    scalar=1.0,
                    in1=mask_all[:, b, :],
                    op0=mybir.AluOpType.mult,
                    op1=mybir.AluOpType.mult,
                    accum_out=cs[:, b:b + 1],
                )
                if prev_chain is not None:
                    add_dep_helper(t_op.ins, prev_chain[1].ins, sync=False,
                                   reason="keep batch chains ordered")
                    add_dep_helper(s_op.ins, prev_chain[2].ins, sync=False,
                                   reason="keep batch chains ordered")
                prev_chain = (t_op, e_op, s_op)

            # ---- global finish: normalize all batches, one out-DMA ----
            # tot[p, b] = sum_p' cs[p', b] (reduce AND broadcast in one
            # matmul with an all-ones stationary); EPS dropped:
            # |eps/denom| ~ 1e-10 << the 2e-2 budget
            tot_ps = psum_small_pool.tile([P, BL], f32, tag="tot")
            nc.tensor.matmul(tot_ps, lhsT=ones_mat, rhs=cs,
                             start=True, stop=True)
            nc.vector.reciprocal(rec, tot_ps)
            rec_bc = bass_mod.AP(
                tensor=rec.tensor, offset=rec.offset,
                ap=[rec.ap[0], rec.ap[1], [0, J]])
            nc.vector.tensor_mul(an, au, rec_bc)
            at_ps = psum_small_pool.tile([BL * J, P], f32, tag="at")
            nc.tensor.transpose(at_ps, an.rearrange("p b j -> p (b j)"),
                                identity32)
            an_t = consts.tile([BL * J, P], f32)
            nc.scalar.copy(an_t, at_ps)
            nc.scalar.dma_start(
                out=out_ext.rearrange("b (j p) -> (b j) p", p=P), in_=an_t)

    nc.compile()
    return nc


def _get_nc():
    if "nc" not in _CACHE:
        _CACHE["nc"] = _build()
    return _CACHE["nc"]


def _in_maps(x, y, mask, M):
    xT16 = np.ascontiguousarray(
        np.asarray(x, dtype=np.float32).astype(np.float16)
        .transpose(0, 2, 1))
    y16 = np.asarray(y, dtype=np.float32).astype(np.float16)
    mask = np.ascontiguousarray(np.asarray(mask, dtype=np.int32))
    MT16 = np.ascontiguousarray(np.asarray(M, dtype=np.float32)
                                .astype(np.float16).T)
    return [
        {
            "xT16": xT16[i * BL:(i + 1) * BL],
            # [p, dc, b]: ysw[p, dc, b] = y[b, dc*128+p]
            "ysw16": np.ascontiguousarray(
                y16[i * BL:(i + 1) * BL].T.reshape(DC, P, BL)
                .transpose(1, 0, 2)),
            "mask": mask[i * BL:(i + 1) * BL],
            "MT16": MT16,
        }
        for i in range(NCORES)
    ]


def kernel(x, y, mask, M, **_ignored):
    from concourse.bass_utils import run_bass_kernel_spmd

    nc = _get_nc()
    res = run_bass_kernel_spmd(nc, _in_maps(x, y, mask, M),
                               core_ids=list(range(NCORES)))
    out = np.concatenate([res.results[i]["out"] for i in range(NCORES)],
                         axis=0)
    return out.astype(np.float32)
